# revision 10
# baseline (speedup 1.0000x reference)
"""Trainium2 Bass kernel for the ragged-classifier problem.

Computation (row splits are uniform: P=256 problems x S=2048 symbols x Q=64
questions):
    logits[p, q] = valid[p] ? sum_s occ[p, q, s] * nanfix(costs[p, s]) : 0
    nanfix(x) = 1.0 where isnan(x) else x
Outputs (logits[P*Q] f32, valid[P] bool passthrough).

Sharding: data-parallel over problems; each of the 8 cores owns 32 contiguous
problems (16.8 MB of occ). No cross-core communication.

Per-core kernel:
  - costs chunk [32, 2048] is NaN-cleaned (is_equal + copy_predicated) and
    multiplied by valid[p] (tensor_scalar) once.
  - occ is streamed as 16 tiles [128, 2048] where the 128 partitions are
    2 problems x 64 questions.
  - For each tile, TensorE broadcasts the 2 problems' costs rows to the 128
    partitions via an outer-product matmul (indicator [2,128]^T @ costs[2,N])
    into PSUM, and VectorE does a fused multiply + free-dim reduce
    (tensor_tensor_reduce) producing 128 logits per tile.
"""

import numpy as np

P = 256
S = 2048
Q = 64
NCORES = 8
PPC = P // NCORES  # 32 problems per core
NT = PPC // 2  # 16 pair-tiles per core

_CACHE = {}


def _modules():
    if "mods" in _CACHE:
        return _CACHE["mods"]
    import sys

    try:
        import concourse.bass as bass  # noqa: F401
    except ImportError:
        sys.path.insert(0, "/opt/trn_rl_repo")

    import concourse.bass as bass
    import concourse.tile as tile
    from concourse import bacc, mybir
    from concourse import bass_utils

    _CACHE["mods"] = (bass, tile, bacc, mybir, bass_utils)
    return _CACHE["mods"]


def _build_nc():
    if "nc" in _CACHE:
        return _CACHE["nc"]
    bass, tile, bacc, mybir, _ = _modules()
    from contextlib import ExitStack

    f32 = mybir.dt.float32
    op = mybir.AluOpType

    nc = bacc.Bacc(
        "TRN2", target_bir_lowering=False, debug=False, num_devices=NCORES
    )
    occ = nc.dram_tensor("occ", [NT, 128, S], f32, kind="ExternalInput").ap()
    costs = nc.dram_tensor("costs", [PPC, S], f32, kind="ExternalInput").ap()
    validf = nc.dram_tensor("validf", [PPC, 1], f32, kind="ExternalInput").ap()
    seld = nc.dram_tensor("sel", [PPC, NT, 128], f32, kind="ExternalInput").ap()
    out = nc.dram_tensor("logits_pt", [128, NT], f32, kind="ExternalOutput").ap()

    with tile.TileContext(nc) as tc, ExitStack() as ctx:
        const = ctx.enter_context(tc.tile_pool(name="const", bufs=1))
        occp = ctx.enter_context(tc.tile_pool(name="occp", bufs=4))
        prodp = ctx.enter_context(tc.tile_pool(name="prodp", bufs=2))
        psump = ctx.enter_context(tc.tile_pool(name="psump", bufs=2, space="PSUM"))
        resp = ctx.enter_context(tc.tile_pool(name="resp", bufs=1))

        # Indicator for the broadcast outer product: column m of sel[:, t, :]^T
        # picks which costs row partition m receives for pair-tile t.
        # Host-provided constant (engine memsets can't target odd partitions).
        sel = const.tile([PPC, NT, 128], f32)
        nc.sync.dma_start(sel[:, :, :], seld)

        costs_raw = const.tile([PPC, S], f32)
        nc.sync.dma_start(costs_raw[:, :], costs)
        vtile = const.tile([PPC, 1], f32)
        nc.sync.dma_start(vtile[:, :], validf)

        # nanfix: eq = (x == x) -> 0.0 at NaN; start from ones and copy the
        # finite entries over, then scale each problem row by valid[p].
        eq32 = const.tile([PPC, S], f32)
        nc.vector.tensor_tensor(
            eq32[:, :], costs_raw[:, :], costs_raw[:, :], op=op.is_equal
        )
        eq = const.tile([PPC, S], mybir.dt.uint8)
        nc.gpsimd.tensor_copy(eq[:, :], eq32[:, :])
        costs_m = const.tile([PPC, S], f32)
        nc.gpsimd.memset(costs_m[:, :], 1.0)
        nc.vector.copy_predicated(costs_m[:, :], eq[:, :], costs_raw[:, :])
        nc.vector.tensor_scalar_mul(costs_m[:, :], costs_m[:, :], vtile[:, :])

        res = resp.tile([128, NT], f32)
        for t in range(NT):
            occ_t = occp.tile([128, S], f32)
            nc.sync.dma_start(occ_t[:, :], occ[t])
            bc = psump.tile([128, S], f32)
            for n in range(4):
                sl = slice(n * 512, (n + 1) * 512)
                nc.tensor.matmul(
                    bc[:, sl],
                    sel[:, t, :],
                    costs_m[:, sl],
                    start=True,
                    stop=True,
                )
            prod = prodp.tile([128, S], f32)
            nc.vector.affine_mul_reduce(
                out=prod[:, :],
                accum_out=res[:, t : t + 1],
                in0=occ_t[:, :],
                in1=bc[:, :],
                scale=1.0,
                bias=0.0,
            )
        nc.sync.dma_start(out, res[:, :])

    nc.compile()
    _CACHE["nc"] = nc
    return nc


def _run(inputs, trace=False, **kw):
    _, _, _, _, bass_utils = _modules()
    nc = _build_nc()

    occ_flat = np.ascontiguousarray(np.asarray(inputs["occ_flat"], dtype=np.float32))
    costs_flat = np.ascontiguousarray(
        np.asarray(inputs["costs_flat"], dtype=np.float32)
    )
    valid = np.asarray(inputs["valid"])

    occ5 = occ_flat.reshape(NCORES, NT, 128, S)
    costs3 = costs_flat.reshape(NCORES, PPC, S)
    validf = valid.reshape(NCORES, PPC, 1).astype(np.float32)

    # sel[k, t, m] = 1 iff problem-within-core k owns partition m of pair-tile
    # t (partitions 0:64 -> problem 2t, 64:128 -> problem 2t+1).
    sel = np.zeros((PPC, NT, 128), dtype=np.float32)
    for t in range(NT):
        sel[2 * t, t, 0:64] = 1.0
        sel[2 * t + 1, t, 64:128] = 1.0

    in_maps = []
    for c in range(NCORES):
        in_maps.append(
            {
                "occ": np.ascontiguousarray(occ5[c]),
                "costs": np.ascontiguousarray(costs3[c]),
                "validf": np.ascontiguousarray(validf[c]),
                "sel": sel,
            }
        )

    results = bass_utils.run_bass_kernel_spmd(
        nc, in_maps, core_ids=list(range(NCORES)), trace=trace, **kw
    )

    logits = np.concatenate(
        [r["logits_pt"].T.reshape(-1) for r in results.results]
    ).astype(np.float32)
    return logits, results


def kernel(**inputs):
    valid = np.asarray(inputs["valid"])
    logits, _ = _run(inputs)
    return logits, valid


# revision 15
# speedup vs baseline: 1.7493x; 1.7493x over previous
"""Trainium2 Bass kernel for the ragged-classifier problem.

Computation (row splits are uniform: P=256 problems x S=2048 symbols x Q=64
questions):
    logits[p, q] = valid[p] ? sum_s occ[p, q, s] * nanfix(costs[p, s]) : 0
    nanfix(x) = 1.0 where isnan(x) else x
Outputs (logits[P*Q] f32, valid[P] bool passthrough).

Sharding: data-parallel over problems; each of the 8 cores owns 32 contiguous
problems (16.8 MB of occ). No cross-core communication.

Per-core kernel:
  - costs chunk [32, 2048] is NaN-cleaned (is_equal + copy_predicated) and
    multiplied by valid[p] (tensor_scalar) once.
  - occ is streamed as 16 tiles [128, 2048] where the 128 partitions are
    2 problems x 64 questions.
  - For each tile, TensorE broadcasts the 2 problems' costs rows to the 128
    partitions via an outer-product matmul (indicator [2,128]^T @ costs[2,N])
    into PSUM, and VectorE does a fused multiply + free-dim reduce
    (tensor_tensor_reduce) producing 128 logits per tile.
"""

import numpy as np

P = 256
S = 2048
Q = 64
NCORES = 8
PPC = P // NCORES  # 32 problems per core
NT = PPC // 2  # 16 pair-tiles per core

_CACHE = {}


def _modules():
    if "mods" in _CACHE:
        return _CACHE["mods"]
    import sys

    try:
        import concourse.bass as bass  # noqa: F401
    except ImportError:
        sys.path.insert(0, "/opt/trn_rl_repo")

    import concourse.bass as bass
    import concourse.tile as tile
    from concourse import bacc, mybir
    from concourse import bass_utils

    _CACHE["mods"] = (bass, tile, bacc, mybir, bass_utils)
    return _CACHE["mods"]


def _build_nc():
    if "nc" in _CACHE:
        return _CACHE["nc"]
    bass, tile, bacc, mybir, _ = _modules()
    from contextlib import ExitStack

    f32 = mybir.dt.float32
    op = mybir.AluOpType

    nc = bacc.Bacc(
        "TRN2", target_bir_lowering=False, debug=False, num_devices=NCORES
    )
    bf16 = mybir.dt.bfloat16
    occ = nc.dram_tensor("occ", [NT, 128, S], f32, kind="ExternalInput").ap()
    costs = nc.dram_tensor("costs", [PPC, S], f32, kind="ExternalInput").ap()
    validf = nc.dram_tensor("validf", [PPC, 1], f32, kind="ExternalInput").ap()
    seld = nc.dram_tensor("sel", [2 * PPC, NT, 128], bf16, kind="ExternalInput").ap()
    out = nc.dram_tensor("logits_pt", [128, NT], f32, kind="ExternalOutput").ap()

    with tile.TileContext(nc) as tc, ExitStack() as ctx:
        const = ctx.enter_context(tc.tile_pool(name="const", bufs=1))
        occp = ctx.enter_context(tc.tile_pool(name="occp", bufs=6))
        prodp = ctx.enter_context(tc.tile_pool(name="prodp", bufs=2))
        psump = ctx.enter_context(tc.tile_pool(name="psump", bufs=2, space="PSUM"))
        resp = ctx.enter_context(tc.tile_pool(name="resp", bufs=1))

        # Indicator for the broadcast outer product: column m of sel[:, t, :]^T
        # picks which costs row partition m receives for pair-tile t. Stacked
        # twice on the K axis so one bf16 matmul sums the hi+lo costs halves.
        # Host-provided constant (engine memsets can't target odd partitions).
        sel = const.tile([2 * PPC, NT, 128], bf16)
        nc.sync.dma_start(sel[:, :, :], seld)

        costs_raw = const.tile([PPC, S], f32)
        nc.sync.dma_start(costs_raw[:, :], costs)
        vtile = const.tile([PPC, 1], f32)
        nc.sync.dma_start(vtile[:, :], validf)

        # nanfix: eq = (x == x) -> 0.0 at NaN; start from ones and copy the
        # finite entries over, then scale each problem row by valid[p].
        eq32 = const.tile([PPC, S], f32)
        nc.vector.tensor_tensor(
            eq32[:, :], costs_raw[:, :], costs_raw[:, :], op=op.is_equal
        )
        eq = const.tile([PPC, S], mybir.dt.uint8)
        nc.gpsimd.tensor_copy(eq[:, :], eq32[:, :])
        costs_m = const.tile([PPC, S], f32)
        nc.gpsimd.memset(costs_m[:, :], 1.0)
        nc.vector.copy_predicated(costs_m[:, :], eq[:, :], costs_raw[:, :])
        nc.vector.tensor_scalar_mul(costs_m[:, :], costs_m[:, :], vtile[:, :])

        # fp32 matmul streams at ~1/4 rate, so split the cleaned costs into
        # bf16 hi+lo halves stacked on partitions [0:32]=hi, [32:64]=lo; the
        # K=64 bf16 matmul against the doubled sel reconstructs fp32-accurate
        # broadcasts (error ~2^-17) at full PE stream rate.
        chl = const.tile([2 * PPC, S], bf16)
        nc.vector.tensor_copy(chl[0:PPC, :], costs_m[:, :])
        chi32 = const.tile([PPC, S], f32)
        nc.vector.tensor_copy(chi32[:, :], chl[0:PPC, :])
        clo32 = const.tile([PPC, S], f32)
        nc.vector.tensor_sub(clo32[:, :], costs_m[:, :], chi32[:, :])
        nc.vector.tensor_copy(chl[PPC : 2 * PPC, :], clo32[:, :])

        res = resp.tile([128, NT], f32)
        for t in range(NT):
            occ_t = occp.tile([128, S], f32)
            dma_eng = nc.sync if t % 2 == 0 else nc.scalar
            dma_eng.dma_start(occ_t[:, :], occ[t])
            bc = psump.tile([128, S], f32)
            for n in range(4):
                sl = slice(n * 512, (n + 1) * 512)
                nc.tensor.matmul(
                    bc[:, sl],
                    sel[:, t, :],
                    chl[:, sl],
                    start=True,
                    stop=True,
                )
            prod = prodp.tile([128, S], f32)
            nc.vector.affine_mul_reduce(
                out=prod[:, :],
                accum_out=res[:, t : t + 1],
                in0=occ_t[:, :],
                in1=bc[:, :],
                scale=1.0,
                bias=0.0,
            )
        nc.sync.dma_start(out, res[:, :])

    nc.compile()
    _CACHE["nc"] = nc
    return nc


def _run(inputs, trace=False, **kw):
    _, _, _, _, bass_utils = _modules()
    nc = _build_nc()

    occ_flat = np.ascontiguousarray(np.asarray(inputs["occ_flat"], dtype=np.float32))
    costs_flat = np.ascontiguousarray(
        np.asarray(inputs["costs_flat"], dtype=np.float32)
    )
    valid = np.asarray(inputs["valid"])

    occ5 = occ_flat.reshape(NCORES, NT, 128, S)
    costs3 = costs_flat.reshape(NCORES, PPC, S)
    validf = valid.reshape(NCORES, PPC, 1).astype(np.float32)

    # sel[k, t, m] = 1 iff problem-within-core k owns partition m of pair-tile
    # t (partitions 0:64 -> problem 2t, 64:128 -> problem 2t+1). Stacked twice
    # on axis 0 to sum the bf16 hi/lo costs halves in one K=64 matmul.
    import ml_dtypes

    sel1 = np.zeros((PPC, NT, 128), dtype=np.float32)
    for t in range(NT):
        sel1[2 * t, t, 0:64] = 1.0
        sel1[2 * t + 1, t, 64:128] = 1.0
    sel = np.concatenate([sel1, sel1], axis=0).astype(ml_dtypes.bfloat16)

    in_maps = []
    for c in range(NCORES):
        in_maps.append(
            {
                "occ": np.ascontiguousarray(occ5[c]),
                "costs": np.ascontiguousarray(costs3[c]),
                "validf": np.ascontiguousarray(validf[c]),
                "sel": sel,
            }
        )

    results = bass_utils.run_bass_kernel_spmd(
        nc, in_maps, core_ids=list(range(NCORES)), trace=trace, **kw
    )

    logits = np.concatenate(
        [r["logits_pt"].T.reshape(-1) for r in results.results]
    ).astype(np.float32)
    return logits, results


def kernel(**inputs):
    valid = np.asarray(inputs["valid"])
    logits, _ = _run(inputs)
    return logits, valid
